# revision 19
# baseline (speedup 1.0000x reference)
"""Trainium2 Bass kernel for nn_LlamaAttention_61495341744411.

Sharding: tensor-parallel over heads across 8 NeuronCores.
  core c: q heads [4c, 4c+4), kv head c, wo cols [512c, 512c+512).
  Each core computes a full-token partial of out^T; host sums partials.

v2 design (per core, single SPMD program):
  - q/k/v projections in fp8(e4m3) DoubleRow with hi/lo error correction:
    X = Xh + Xl/32, W' = 32W = Wh + Wl;  W'X ~= Wh.Xh + Wl.Xh + (Wh/32).Xl
    computed as 3 DoubleRow matmuls per 2 k-tiles (1.33x bf16 FLOP rate,
    near-bf16 accuracy; validated vs reference in numpy).  The /32
    prescale is folded into the RoPE cos/sin tables (q,k) and the v copy.
  - attention computed in score-transposed orientation (S^T = K^T.q panels
    of 512), eliminating all P-transposes; PV accumulates oT directly;
    softmax denominators via ones-vector matmul; normalization by a
    rank-1 PE broadcast of 1/denom + one DVE multiply per (panel, head).
  - kv-cache K is RoPE'd on the host; decode attention is interleaved
    across the prefill panels so its DMA fully overlaps compute.
  - o_proj per panel from SBUF-resident attn outputs (no DRAM spills).
"""
import sys

if "/opt/trn_rl_repo" not in sys.path:
    sys.path.insert(0, "/opt/trn_rl_repo")

import numpy as np
import ml_dtypes

BF16 = ml_dtypes.bfloat16
E4M3 = ml_dtypes.float8_e4m3

PREFILLS = [1024, 1536, 2048, 512]
DOFF = sum(PREFILLS)            # 5120
DECODE = 32
PAST = 2048
HIDDEN = 4096
NQ, NKV, HD = 32, 8, 128
G = NQ // NKV                   # 4
T = DOFF + DECODE               # 5152
SCALE = 1.0 / float(np.sqrt(HD))
NCORES = 8
QH = NQ // NCORES               # 4 q heads per core
ADIM = QH * HD                  # 512
KS = HIDDEN // 128              # 32 contraction k-tiles
P = 128
TW = 256                        # projection token-tile width
PW = 512                        # attention q-panel width
NT = (T + TW - 1) // TW         # 21 token tiles (last = decode, 32 valid)
NKT_D = PAST // P               # 16 decode cache k-tiles
WS = 32.0                       # weight prescale

SEQ_BOUNDS = []
_off = 0
for _L in PREFILLS:
    SEQ_BOUNDS.append((_off, _L))
    _off += _L

# (si, panel, global t0) for every 512-token prefill panel.
# Short seq first so the largest panel (max ILP) lands last and hides
# the decode/o_proj tail.
SEQ_ORDER = [3, 0, 1, 2]
PANELS = []
for _si in SEQ_ORDER:
    _s0, _L = SEQ_BOUNDS[_si]
    for _p in range(_L // PW):
        PANELS.append((_si, _p, _s0 + _p * PW))


def build_program():
    import concourse.mybir as mybir
    import concourse.tile as tile
    from concourse import bacc
    from concourse.masks import make_identity
    from contextlib import ExitStack

    dt = mybir.dt
    AF = mybir.ActivationFunctionType
    ALU = mybir.AluOpType
    DR = mybir.MatmulPerfMode.DoubleRow
    f32 = dt.float32
    bf = dt.bfloat16
    f8 = dt.float8e4

    nc = bacc.Bacc(None, target_bir_lowering=False, debug=False)

    ht8 = nc.dram_tensor("ht8", [NT, P, KS, 2, TW], f8, kind="ExternalInput")
    whq = nc.dram_tensor("whq", [P, KS, ADIM], f8, kind="ExternalInput")
    wcq = nc.dram_tensor("wcq", [P, KS, 2, ADIM], f8, kind="ExternalInput")
    whk = nc.dram_tensor("whk", [P, KS, HD], f8, kind="ExternalInput")
    wck = nc.dram_tensor("wck", [P, KS, 2, HD], f8, kind="ExternalInput")
    whv = nc.dram_tensor("whv", [P, KS, HD], f8, kind="ExternalInput")
    wcv = nc.dram_tensor("wcv", [P, KS, 2, HD], f8, kind="ExternalInput")
    woh = nc.dram_tensor("woh", [P, QH, HIDDEN], f8, kind="ExternalInput")
    wol = nc.dram_tensor("wol", [P, QH, HIDDEN], f8, kind="ExternalInput")
    kTc = nc.dram_tensor("kTc", [DECODE, HD, PAST], bf, kind="ExternalInput")
    vcn = nc.dram_tensor("vcn", [DECODE, P, NKT_D, HD + 1], bf,
                         kind="ExternalInput")
    qcos = nc.dram_tensor("qcos", [HD, T], bf, kind="ExternalInput")
    qsin = nc.dram_tensor("qsin", [HD, T], bf, kind="ExternalInput")
    outT = nc.dram_tensor("outT", [HIDDEN, T], bf, kind="ExternalOutput")
    outT_r = outT.rearrange("(o p) t -> p o t", p=P)    # [128, 32, T]

    with ExitStack() as ctx:
        tc = ctx.enter_context(tile.TileContext(nc))
        p1 = ctx.enter_context(tc.tile_pool(name="p1", bufs=1))
        pseq = ctx.enter_context(tc.tile_pool(name="pseq", bufs=2))
        ppan = ctx.enter_context(tc.tile_pool(name="ppan", bufs=2))
        pht = ctx.enter_context(tc.tile_pool(name="pht", bufs=2))
        ppt = ctx.enter_context(tc.tile_pool(name="ppt", bufs=3))
        pdec = ctx.enter_context(tc.tile_pool(name="pdec", bufs=1))
        pvd = ctx.enter_context(tc.tile_pool(name="pvd", bufs=1))
        pd1 = ctx.enter_context(tc.tile_pool(name="pd1", bufs=1))
        pmb = ctx.enter_context(tc.tile_pool(name="pmb", bufs=1))
        psm = ctx.enter_context(tc.tile_pool(name="psm", bufs=2))
        psS = ctx.enter_context(tc.tile_pool(name="psS", bufs=2, space="PSUM"))
        psO = ctx.enter_context(tc.tile_pool(name="psO", bufs=2, space="PSUM"))
        psD = ctx.enter_context(tc.tile_pool(name="psD", bufs=1, space="PSUM"))
        psA = ctx.enter_context(tc.tile_pool(name="psA", bufs=3, space="PSUM"))

        ident = p1.tile([P, P], bf, tag="ident")
        make_identity(nc, ident)
        ones_c = p1.tile([P, 1], bf, tag="ones_c")
        nc.vector.memset(ones_c[:], 1.0)
        ones_r = p1.tile([1, P], bf, tag="ones_r")
        nc.vector.memset(ones_r[:], 1.0)

        # ---- weight tiles (DMAs emitted in the startup sequence below,
        # interleaved with the first ht tile so PE starts early) ----
        wk_h = p1.tile([P, KS, HD], f8, tag="wk_h")
        wk_c = p1.tile([P, KS, 2, HD], f8, tag="wk_c")
        wv_h = p1.tile([P, KS, HD], f8, tag="wv_h")
        wv_c = p1.tile([P, KS, 2, HD], f8, tag="wv_c")
        wq_h = p1.tile([P, KS, ADIM], f8, tag="wq_h")
        wq_c = p1.tile([P, KS, 2, ADIM], f8, tag="wq_c")
        wo_h = p1.tile([P, QH, HIDDEN], f8, tag="wo_h")
        wo_l = p1.tile([P, QH, HIDDEN], f8, tag="wo_l")

        ht_cache = {}

        def load_ht(ti):
            ht = pht.tile([P, KS, 2, TW], f8, tag="ht")
            nc.sync.dma_start(ht[:], ht8[ti])
            ht_cache[ti] = ht
            return ht

        # per-seq resident k / v (rotating, sized for the longest seq)
        kT_seq = {}
        v_seq = {}

        # decode persistent tiles
        kT_dec = p1.tile([P, DECODE], bf, tag="kTdec")
        qdec_t = p1.tile([P, QH, DECODE], bf, tag="qdect")
        qdec_sb = p1.tile([P, P], bf, tag="qdec")
        vdt = p1.tile([DECODE, HD], bf, tag="vdt")
        odec_sb = p1.tile([P, HD], bf, tag="odec")
        aT_dec_h = p1.tile([P, QH, DECODE], f8, tag="aTdech")
        aT_dec_l = p1.tile([P, QH, DECODE], f8, tag="aTdecl")

        def proj_block(ps, wh, wc, ht, W):
            """fp8 DoubleRow projection of one 128-wide output block."""
            for j in range(KS // 2):
                nc.tensor.matmul(
                    ps[:, :W], lhsT=wh[:, 2 * j:2 * j + 2, :],
                    rhs=ht[:, 2 * j:2 * j + 2, 0, :W],
                    start=(j == 0), stop=False, perf_mode=DR)
            for kt in range(KS):
                nc.tensor.matmul(
                    ps[:, :W], lhsT=wc[:, kt, :, :],
                    rhs=ht[:, kt, :, :W],
                    start=False, stop=(kt == KS - 1), perf_mode=DR)

        def ph1_tile(ti, W, kT_dst, kcol0, v_dst, q_dst, qcol0):
            """Projections + rope for token tile ti (W valid cols).
            kT_dst[:, kcol0:+W] gets roped k;  q_dst [P, QH, *] gets roped
            q at qcol0;  v_dst: prefill -> v_nat [P, kt, HD+1] at k-tile
            kcol0//P (W=256), decode -> vdt [DECODE, HD] (W=32)."""
            t0 = ti * TW
            ht = ht_cache.pop(ti, None)
            if ht is None:
                ht = load_ht(ti)
                ht_cache.pop(ti)
            ct = pht.tile([P, TW], bf, tag="cos")
            st = pht.tile([P, TW], bf, tag="sin")
            nc.sync.dma_start(ct[:, :W], qcos[:, t0:t0 + W])
            nc.sync.dma_start(st[:, :W], qsin[:, t0:t0 + W])

            NB = QH + 1
            xq = pht.tile([P, NB, TW], bf, tag="xq")
            # k first (weights arrive first), then v, then q heads
            ps = psA.tile([P, 512], f32, tag="psA")
            proj_block(ps, wk_h, wk_c, ht, W)
            nc.scalar.activation(xq[:, QH, :W], ps[:, :W], AF.Copy)

            ps = psA.tile([P, 512], f32, tag="psA")
            proj_block(ps, wv_h, wv_c, ht, W)
            vt = pht.tile([P, TW], bf, tag="vt")
            nc.scalar.activation(vt[:, :W], ps[:, :W], AF.Copy,
                                 scale=1.0 / WS)
            if W == TW:
                for j in range(TW // P):
                    pst = psA.tile([P, P], bf, tag="psA")
                    nc.tensor.transpose(pst[:], vt[:, j * P:(j + 1) * P],
                                        ident[:])
                    nc.vector.tensor_copy(
                        out=v_dst[:, kcol0 // P + j, :HD], in_=pst[:])
            else:  # decode tile: W == 32
                pst = psA.tile([P, P], bf, tag="psA")
                nc.tensor.transpose(pst[:W, :], vt[:, :W], ident[:])
                nc.vector.tensor_copy(out=v_dst[:], in_=pst[:W, :])

            for m in range(QH):
                ps = psA.tile([P, 512], f32, tag="psA")
                proj_block(ps, wq_h[:, :, m * P:(m + 1) * P],
                           wq_c[:, :, :, m * P:(m + 1) * P], ht, W)
                nc.scalar.activation(xq[:, m, :W], ps[:, :W], AF.Copy)

            rotq = pht.tile([P, NB, TW], bf, tag="rotq")
            nc.gpsimd.dma_start(out=rotq[0:64, :, :W], in_=xq[64:128, :, :W])
            nc.gpsimd.dma_start(out=rotq[64:128, :, :W], in_=xq[0:64, :, :W])
            ct_b = ct[:, None, :W].to_broadcast((P, NB, W))
            st_b = st[:, None, :W].to_broadcast((P, NB, W))
            nc.vector.tensor_tensor(xq[:, :, :W], xq[:, :, :W], ct_b, ALU.mult)
            nc.vector.tensor_tensor(rotq[:, :, :W], rotq[:, :, :W], st_b,
                                    ALU.mult)
            nc.vector.tensor_tensor(q_dst[:, :, qcol0:qcol0 + W],
                                    xq[:, :QH, :W], rotq[:, :QH, :W], ALU.add)
            nc.vector.tensor_tensor(kT_dst[:, kcol0:kcol0 + W],
                                    xq[:, QH, :W], rotq[:, QH, :W], ALU.add)

        def ph2_panel(si, p, h, qT, aT_h, aT_l):
            """Attention for (seq si, panel p, head h): S^T orientation."""
            kT_sb = kT_seq[si]
            v_nat = v_seq[si]
            nck = 4 * p + 4
            oT_ps = psO.tile([P, PW], f32, tag="psO")
            den = psD.tile([1, PW], f32, tag="psD")

            stage = []   # chunks with pending den+PV (software pipeline)

            def drain_one():
                ct_, off_, w_, pt_ = stage.pop(0)
                nc.tensor.matmul(den[0:1, off_:off_ + w_],
                                 lhsT=ones_c[:], rhs=pt_[:, off_:off_ + w_],
                                 start=(ct_ == 0), stop=(ct_ == nck - 1))
                nc.tensor.matmul(oT_ps[:, off_:off_ + w_],
                                 lhsT=v_nat[:, ct_, :HD],
                                 rhs=pt_[:, off_:off_ + w_],
                                 start=(ct_ == 0), stop=(ct_ == nck - 1))

            for ct in range(nck):
                off = max(0, (ct - 4 * p)) * P
                w = PW - off
                sps = psS.tile([P, PW], f32, tag="psS")
                nc.tensor.matmul(sps[:, off:off + w],
                                 lhsT=kT_sb[:, ct * P:(ct + 1) * P],
                                 rhs=qT[:, h, off:PW],
                                 start=True, stop=True)
                pt = ppt.tile([P, PW], bf, tag="pt")
                nc.scalar.activation(pt[:, off:off + w], sps[:, off:off + w],
                                     AF.Exp, scale=SCALE)
                if ct >= 4 * p:   # diagonal block: mask c > q within 128 cols
                    nc.gpsimd.affine_select(
                        out=pt[:, off:off + P], in_=pt[:, off:off + P],
                        compare_op=ALU.is_ge, fill=0.0,
                        base=0, channel_multiplier=-1, pattern=[[1, P]])
                stage.append((ct, off, w, pt))
                if len(stage) >= 2:
                    drain_one()
            while stage:
                drain_one()

            # normalize: rr = 1/den, partition-broadcast via SWDGE, then
            # aT[:, h, :] = oT * rr_b  (single DVE op)
            rr = psm.tile([1, PW], bf, tag="rr")
            with nc.allow_low_precision(reason="softmax denom bcast in bf16"):
                nc.vector.reciprocal(rr[:], den[0:1, :])
            rr_ps = psA.tile([P, PW], f32, tag="psA")
            nc.tensor.matmul(rr_ps[:], lhsT=ones_r[:], rhs=rr[:],
                             start=True, stop=True)
            rrb = psm.tile([P, PW], bf, tag="rrb")
            nc.scalar.activation(rrb[:], rr_ps[:], AF.Copy)
            # normalized out -> bf16 t1 -> fp8 hi + residual lo
            t1 = pd1.tile([P, PW], bf, tag="t1")
            nc.vector.tensor_tensor(t1[:], oT_ps[:], rrb[:], ALU.mult)
            nc.scalar.activation(aT_h[:, h, :], t1[:], AF.Copy)
            nc.vector.tensor_tensor(aT_l[:, h, :], t1[:], aT_h[:, h, :],
                                    ALU.subtract)

        def ph4_panel(aT_h, aT_l, t0, W):
            """fp8 o_proj partial for W tokens at global t0.
            Wo'X ~= Wh.Xh + Wh.Xl + Wl.Xh, 3 DoubleRows per head-pair,
            emitted in <=256-col halves (DoubleRow moving limit)."""
            halves = [(c0, min(TW, W - c0)) for c0 in range(0, W, TW)]
            for g in range(8):
                omb = pmb.tile([P, 4, PW], bf, tag="omb")
                for mi in range(4):
                    m = g * 4 + mi
                    ms = slice(m * P, (m + 1) * P)
                    ps = psA.tile([P, 512], f32, tag="psA")
                    for c0, cw in halves:
                        cs = slice(c0, c0 + cw)
                        first = True
                        for wsb, asb in ((wo_h, aT_h), (wo_h, aT_l),
                                         (wo_l, aT_h)):
                            for hp in (0, 2):
                                nc.tensor.matmul(
                                    ps[:, cs],
                                    lhsT=wsb[:, hp:hp + 2, ms],
                                    rhs=asb[:, hp:hp + 2, cs],
                                    start=first,
                                    stop=(wsb is wo_l and hp == 2),
                                    perf_mode=DR)
                                first = False
                        if mi % 2 == 0:
                            nc.scalar.activation(omb[:, mi, cs], ps[:, cs],
                                                 AF.Copy, scale=1.0 / WS)
                        else:
                            nc.vector.tensor_scalar(
                                out=omb[:, mi, cs], in0=ps[:, cs],
                                scalar1=1.0 / WS, scalar2=None,
                                op0=ALU.mult)
                nc.sync.dma_start(
                    outT_r[:, g * 4:(g + 1) * 4, t0:t0 + W], omb[:, :, :W])

        # ---------------- decode helpers ----------------
        dec_tiles = {}

        def decode_dma(s):
            kd = pdec.tile([P, PAST], bf, tag="kd")
            nc.sync.dma_start(kd[:], kTc[s])
            vd = pvd.tile([P, NKT_D + 1, HD + 1], bf, tag="vd")
            nc.sync.dma_start(vd[:, :NKT_D, :], vcn[s])
            dec_tiles[s] = (kd, vd)

        def decode_compute(s):
            kd, vd = dec_tiles.pop(s)
            nc.gpsimd.dma_start(out=vd[0:1, NKT_D, :HD], in_=vdt[s:s + 1, :])
            nc.vector.memset(vd[0:1, NKT_D, HD:HD + 1], 1.0)

            stp = psS.tile([P, PW], f32, tag="psS")
            for kt in range(NKT_D):
                nc.tensor.matmul(
                    stp[:, kt * QH:(kt + 1) * QH],
                    lhsT=kd[:, kt * P:(kt + 1) * P],
                    rhs=qdec_sb[:, s * QH:(s + 1) * QH], start=True, stop=True)
            nc.tensor.matmul(
                stp[0:1, 64:68], lhsT=kT_dec[:, s:s + 1],
                rhs=qdec_sb[:, s * QH:(s + 1) * QH], start=True, stop=True)
            pt = pd1.tile([P, PW], bf, tag="ptd")
            nc.scalar.activation(pt[:, :64], stp[:, :64], AF.Exp, scale=SCALE)
            nc.scalar.activation(pt[0:1, 64:68], stp[0:1, 64:68], AF.Exp,
                                 scale=SCALE)

            ov = psA.tile([QH, HD + 1], f32, tag="psA")
            for kt in range(NKT_D):
                nc.tensor.matmul(
                    ov[:], lhsT=pt[:, kt * QH:(kt + 1) * QH],
                    rhs=vd[:, kt, :], start=(kt == 0), stop=False)
            nc.tensor.matmul(ov[:], lhsT=pt[0:1, 64:68],
                             rhs=vd[0:1, NKT_D, :], start=False, stop=True)
            r4 = pd1.tile([QH, 1], f32, tag="r4")
            nc.vector.reciprocal(r4[:], ov[:, HD:HD + 1])
            o4 = pd1.tile([QH, HD], bf, tag="o4")
            nc.vector.tensor_scalar_mul(o4[:], ov[:, :HD], r4[:])
            nc.gpsimd.dma_start(out=odec_sb[s * QH:(s + 1) * QH, :], in_=o4[:])

        # ================= emission =================
        def seq_tiles(si, p):
            s0, L = SEQ_BOUNDS[si]
            kT_sb = kT_seq[si]
            v_nat = v_seq[si]
            qT = qT_pan[(si, p)]
            for half in range(2):
                t0 = s0 + p * PW + half * TW
                ph1_tile(t0 // TW, TW, kT_sb, p * PW + half * TW,
                         v_nat, qT, half * TW)

        qT_pan = {}

        def alloc_seq(si):
            kT_seq[si] = pseq.tile([P, 2048], bf, tag="kT",
                                   name=f"kT{si}")
            v_seq[si] = pseq.tile([P, 2048 // P, HD + 1], bf, tag="vn",
                                  name=f"vn{si}")

        def alloc_panel(si, p):
            qT_pan[(si, p)] = ppan.tile([P, QH, PW], bf, tag="qT",
                                        name=f"qT{si}_{p}")

        # ---- startup: first ht tile DMA, then weights in need-order ----
        si0, p0, _ = PANELS[0]
        s00, _ = SEQ_BOUNDS[si0]
        first_ti = (s00 + p0 * PW) // TW
        load_ht(first_ti)
        nc.sync.dma_start(wk_h[:], whk[:])
        nc.sync.dma_start(wk_c[:], wck[:])
        nc.sync.dma_start(wv_h[:], whv[:])
        nc.sync.dma_start(wv_c[:], wcv[:])
        nc.sync.dma_start(wq_h[:], whq[:])
        load_ht(first_ti + 1)
        nc.sync.dma_start(wq_c[:], wcq[:])

        alloc_seq(si0)
        alloc_panel(si0, p0)
        seq_tiles(si0, p0)
        nc.sync.dma_start(wo_h[:], woh[:])
        nc.sync.dma_start(wo_l[:], wol[:])
        ph1_tile(NT - 1, DECODE, kT_dec, 0, vdt, qdec_t, 0)
        qd_r = qdec_sb.rearrange("p (s h) -> p s h", h=QH)
        for h in range(QH):
            nc.gpsimd.dma_start(out=qd_r[:, :, h], in_=qdec_t[:, h, :])

        dec_next_dma = 0
        dec_next_cmp = 0
        dec_out_done = False

        def decode_outputs():
            # decode outputs -> aT_dec hi/lo -> o_proj
            pst = psA.tile([P, P], bf, tag="psA")
            nc.tensor.transpose(pst[:], odec_sb[:], ident[:])
            ot = pd1.tile([P, P], bf, tag="otd")
            nc.vector.tensor_copy(out=ot[:], in_=pst[:])
            ot_h = pd1.tile([P, P], f8, tag="otdh")
            nc.scalar.activation(ot_h[:], ot[:], AF.Copy)
            ot_l = pd1.tile([P, P], f8, tag="otdl")
            nc.vector.tensor_tensor(ot_l[:], ot[:], ot_h[:], ALU.subtract)
            oh_r = ot_h.rearrange("d (s h) -> d s h", h=QH)
            ol_r = ot_l.rearrange("d (s h) -> d s h", h=QH)
            for h in range(QH):
                nc.gpsimd.dma_start(out=aT_dec_h[:, h, :], in_=oh_r[:, :, h])
                nc.gpsimd.dma_start(out=aT_dec_l[:, h, :], in_=ol_r[:, :, h])
            ph4_panel(aT_dec_h, aT_dec_l, DOFF, DECODE)

        def decode_slot():
            nonlocal dec_next_dma, dec_next_cmp, dec_out_done
            if dec_next_cmp < dec_next_dma:
                decode_compute(dec_next_cmp)
                dec_next_cmp += 1
            if dec_next_dma < DECODE:
                decode_dma(dec_next_dma)
                dec_next_dma += 1
            if dec_next_cmp == DECODE and not dec_out_done:
                dec_out_done = True
                decode_outputs()

        for step, (si, p, t0g) in enumerate(PANELS):
            # ph1 one panel ahead
            if step + 1 < len(PANELS):
                nsi, np_, _ = PANELS[step + 1]
                if np_ == 0:
                    alloc_seq(nsi)
                alloc_panel(nsi, np_)
                seq_tiles(nsi, np_)
            aT_h = ppan.tile([P, QH, PW], f8, tag="aTh", name=f"aTh{si}_{p}")
            aT_l = ppan.tile([P, QH, PW], f8, tag="aTl", name=f"aTl{si}_{p}")
            qT = qT_pan.pop((si, p))
            for h in range(QH):
                ph2_panel(si, p, h, qT, aT_h, aT_l)
                decode_slot()
            ph4_panel(aT_h, aT_l, t0g, PW)
            decode_slot()

        # remaining decode (normally already drained)
        while dec_next_cmp < DECODE:
            if dec_next_dma < DECODE:
                decode_dma(dec_next_dma)
                dec_next_dma += 1
            decode_compute(dec_next_cmp)
            dec_next_cmp += 1
        if not dec_out_done:
            decode_outputs()

    nc.compile()
    return nc


_NC = None


def _get_program():
    global _NC
    if _NC is None:
        _NC = build_program()
    return _NC


def _rope_tables():
    """cos/sin tables [128, T] with the 1/WS projection descale folded in."""
    inv_freq = 1.0 / (10000.0 ** (np.arange(0, HD, 2, dtype=np.float32) / HD))
    pos_q = np.concatenate(
        [np.arange(L, dtype=np.float32) for L in PREFILLS]
        + [np.full(DECODE, float(PAST), np.float32)])                 # [T]
    ang = np.outer(inv_freq, pos_q)                                   # [64, T]
    qcos = np.concatenate([np.cos(ang), np.cos(ang)], axis=0) / WS
    qsin = np.concatenate([-np.sin(ang), np.sin(ang)], axis=0) / WS
    return qcos.astype(BF16), qsin.astype(BF16)


def _split_w(wT):
    """wT [4096, M] f32 -> (wh [128, 32, M], wc [128, 32, 2, M]) fp8."""
    M = wT.shape[1]
    wp = (wT * WS).astype(np.float32)
    wh = wp.astype(E4M3)
    wl = (wp - wh.astype(np.float32)).astype(E4M3)
    wd = (wh.astype(np.float32) / WS).astype(E4M3)
    wh3 = np.ascontiguousarray(
        wh.reshape(KS, P, M).transpose(1, 0, 2))
    wc = np.ascontiguousarray(
        np.stack([wl.reshape(KS, P, M), wd.reshape(KS, P, M)],
                 axis=2).transpose(1, 0, 2, 3))
    return wh3, wc


def make_in_maps(hidden_states, wq, wk, wv, wo, kv_cache_k, kv_cache_v):
    hidden_states = np.asarray(hidden_states, np.float32)
    wq, wk, wv, wo = (np.asarray(a, np.float32) for a in (wq, wk, wv, wo))
    kv_cache_k = np.asarray(kv_cache_k, np.float32)
    kv_cache_v = np.asarray(kv_cache_v, np.float32)

    # hidden^T split into fp8 hi/lo, packed tile-major [NT,128,32,2,256]
    hT = hidden_states.T                                   # [4096, T]
    pad = NT * TW - T
    hTp = np.pad(hT, ((0, 0), (0, pad)))
    xh = hTp.astype(E4M3)
    xl = ((hTp - xh.astype(np.float32)) * WS).astype(E4M3)
    # [4096, NTT] -> [32, 128, NT, 256] -> [NT, 128, 32, 256]
    def pack(a):
        return a.reshape(KS, P, NT, TW).transpose(2, 1, 0, 3)
    ht8 = np.ascontiguousarray(
        np.stack([pack(xh), pack(xl)], axis=3))            # [NT,128,32,2,256]

    qcos, qsin = _rope_tables()

    # host-side RoPE of the k cache (reference semantics, fp32)
    inv_freq = 1.0 / (10000.0 ** (np.arange(0, HD, 2, dtype=np.float32) / HD))
    kpos = np.arange(PAST, dtype=np.float32)
    ang = np.outer(kpos, inv_freq)                          # [PAST, 64]
    cos = np.concatenate([np.cos(ang), np.cos(ang)], axis=1)[None, :, None, :]
    sin = np.concatenate([np.sin(ang), np.sin(ang)], axis=1)[None, :, None, :]
    rot = np.concatenate([-kv_cache_k[..., HD // 2:],
                          kv_cache_k[..., :HD // 2]], axis=-1)
    kroped = kv_cache_k * cos + rot * sin                   # [D, PAST, 8, HD]

    in_maps = []
    for c in range(NCORES):
        wh_q, wc_q = _split_w(wq[c * ADIM:(c + 1) * ADIM, :].T)
        wh_k, wc_k = _split_w(wk[c * HD:(c + 1) * HD, :].T)
        wh_v, wc_v = _split_w(wv[c * HD:(c + 1) * HD, :].T)
        wot = wo[:, c * ADIM:(c + 1) * ADIM].T.reshape(QH, P, HIDDEN) \
            .transpose(1, 0, 2) * WS                        # [128, 4, 4096]
        wo_hq = wot.astype(E4M3)
        wo_lq = (wot - wo_hq.astype(np.float32)).astype(E4M3)
        wo_hq = np.ascontiguousarray(wo_hq)
        wo_lq = np.ascontiguousarray(wo_lq)
        kTcc = np.ascontiguousarray(
            kroped[:, :, c, :].transpose(0, 2, 1).astype(BF16))  # [D,128,PAST]
        # v cache -> [D, 128, 16, 129] with ones column baked in
        vcc = kv_cache_v[:, :, c, :].reshape(DECODE, NKT_D, P, HD)
        vcc = vcc.transpose(0, 2, 1, 3)                     # [D, 128, 16, HD]
        vcn = np.concatenate(
            [vcc, np.ones((DECODE, P, NKT_D, 1), np.float32)], axis=3)
        vcn = np.ascontiguousarray(vcn.astype(BF16))
        in_maps.append({
            "ht8": ht8, "whq": wh_q, "wcq": wc_q, "whk": wh_k, "wck": wc_k,
            "whv": wh_v, "wcv": wc_v, "woh": wo_hq, "wol": wo_lq, "kTc": kTcc, "vcn": vcn,
            "qcos": qcos, "qsin": qsin,
        })
    return in_maps


def combine_outputs(results):
    acc = np.zeros((HIDDEN, T), np.float32)
    for c in range(NCORES):
        acc += results[c]["outT"].astype(np.float32)
    return np.ascontiguousarray(acc.T)


def kernel(hidden_states, wq, wk, wv, wo, kv_cache_k, kv_cache_v):
    from concourse.bass_utils import run_bass_kernel_spmd

    nc = _get_program()
    in_maps = make_in_maps(hidden_states, wq, wk, wv, wo, kv_cache_k,
                           kv_cache_v)
    res = run_bass_kernel_spmd(nc, in_maps, core_ids=list(range(NCORES)))
    return combine_outputs(res.results)


# revision 22
# speedup vs baseline: 1.2035x; 1.2035x over previous
"""Trainium2 Bass kernel for nn_LlamaAttention_61495341744411.

Sharding: tensor-parallel over heads across 8 NeuronCores.
  core c: q heads [4c, 4c+4), kv head c, wo cols [512c, 512c+512).
  Each core computes a full-token partial of out^T; host sums partials.

v2 design (per core, single SPMD program):
  - q/k/v projections in fp8(e4m3) DoubleRow with hi/lo error correction:
    X = Xh + Xl/32, W' = 32W = Wh + Wl;  W'X ~= Wh.Xh + Wl.Xh + (Wh/32).Xl
    computed as 3 DoubleRow matmuls per 2 k-tiles (1.33x bf16 FLOP rate,
    near-bf16 accuracy; validated vs reference in numpy).  The /32
    prescale is folded into the RoPE cos/sin tables (q,k) and the v copy.
  - attention computed in score-transposed orientation (S^T = K^T.q panels
    of 512), eliminating all P-transposes; PV accumulates oT directly;
    softmax denominators via ones-vector matmul; normalization by a
    rank-1 PE broadcast of 1/denom + one DVE multiply per (panel, head).
  - kv-cache K is RoPE'd on the host; decode attention is interleaved
    across the prefill panels so its DMA fully overlaps compute.
  - o_proj per panel from SBUF-resident attn outputs (no DRAM spills).
"""
import sys

if "/opt/trn_rl_repo" not in sys.path:
    sys.path.insert(0, "/opt/trn_rl_repo")

import numpy as np
import ml_dtypes

BF16 = ml_dtypes.bfloat16
E4M3 = ml_dtypes.float8_e4m3

PREFILLS = [1024, 1536, 2048, 512]
DOFF = sum(PREFILLS)            # 5120
DECODE = 32
PAST = 2048
HIDDEN = 4096
NQ, NKV, HD = 32, 8, 128
G = NQ // NKV                   # 4
T = DOFF + DECODE               # 5152
SCALE = 1.0 / float(np.sqrt(HD))
NCORES = 8
QH = NQ // NCORES               # 4 q heads per core
ADIM = QH * HD                  # 512
KS = HIDDEN // 128              # 32 contraction k-tiles
P = 128
TW = 256                        # projection token-tile width
PW = 512                        # attention q-panel width
NT = (T + TW - 1) // TW         # 21 token tiles (last = decode, 32 valid)
NKT_D = PAST // P               # 16 decode cache k-tiles
WS = 32.0                       # weight prescale

SEQ_BOUNDS = []
_off = 0
for _L in PREFILLS:
    SEQ_BOUNDS.append((_off, _L))
    _off += _L

# (si, panel, global t0) for every 512-token prefill panel.
# Short seq first so the largest panel (max ILP) lands last and hides
# the decode/o_proj tail.
SEQ_ORDER = [3, 0, 1, 2]
PANELS = []
for _si in SEQ_ORDER:
    _s0, _L = SEQ_BOUNDS[_si]
    for _p in range(_L // PW):
        PANELS.append((_si, _p, _s0 + _p * PW))


def build_program():
    import concourse.mybir as mybir
    import concourse.tile as tile
    from concourse import bacc
    from concourse.masks import make_identity
    from contextlib import ExitStack

    dt = mybir.dt
    AF = mybir.ActivationFunctionType
    ALU = mybir.AluOpType
    DR = mybir.MatmulPerfMode.DoubleRow
    f32 = dt.float32
    bf = dt.bfloat16
    f8 = dt.float8e4

    nc = bacc.Bacc(None, target_bir_lowering=False, debug=False)

    ht8 = nc.dram_tensor("ht8", [NT, P, KS, 2, TW], f8, kind="ExternalInput")
    whq = nc.dram_tensor("whq", [P, KS, ADIM], f8, kind="ExternalInput")
    wcq = nc.dram_tensor("wcq", [P, KS, 2, ADIM], f8, kind="ExternalInput")
    whk = nc.dram_tensor("whk", [P, KS, HD], f8, kind="ExternalInput")
    wck = nc.dram_tensor("wck", [P, KS, 2, HD], f8, kind="ExternalInput")
    whv = nc.dram_tensor("whv", [P, KS, HD], f8, kind="ExternalInput")
    wcv = nc.dram_tensor("wcv", [P, KS, 2, HD], f8, kind="ExternalInput")
    woh = nc.dram_tensor("woh", [P, QH, HIDDEN], f8, kind="ExternalInput")
    wol = nc.dram_tensor("wol", [P, QH, HIDDEN], f8, kind="ExternalInput")
    kTc = nc.dram_tensor("kTc", [DECODE, HD, PAST], bf, kind="ExternalInput")
    vcn = nc.dram_tensor("vcn", [DECODE, P, NKT_D, HD + 1], bf,
                         kind="ExternalInput")
    qcos = nc.dram_tensor("qcos", [HD, T], bf, kind="ExternalInput")
    qsin = nc.dram_tensor("qsin", [HD, T], bf, kind="ExternalInput")
    outT = nc.dram_tensor("outT", [HIDDEN, T], bf, kind="ExternalOutput")
    outT_r = outT.rearrange("(o p) t -> p o t", p=P)    # [128, 32, T]

    with ExitStack() as ctx:
        tc = ctx.enter_context(tile.TileContext(nc))
        p1 = ctx.enter_context(tc.tile_pool(name="p1", bufs=1))
        pseq = ctx.enter_context(tc.tile_pool(name="pseq", bufs=2))
        ppan = ctx.enter_context(tc.tile_pool(name="ppan", bufs=2))
        pht = ctx.enter_context(tc.tile_pool(name="pht", bufs=2))
        ppt = ctx.enter_context(tc.tile_pool(name="ppt", bufs=3))
        pdec = ctx.enter_context(tc.tile_pool(name="pdec", bufs=1))
        pvd = ctx.enter_context(tc.tile_pool(name="pvd", bufs=1))
        pd1 = ctx.enter_context(tc.tile_pool(name="pd1", bufs=1))
        pmb = ctx.enter_context(tc.tile_pool(name="pmb", bufs=2))
        psm = ctx.enter_context(tc.tile_pool(name="psm", bufs=2))
        psS = ctx.enter_context(tc.tile_pool(name="psS", bufs=3, space="PSUM"))
        psO = ctx.enter_context(tc.tile_pool(name="psO", bufs=2, space="PSUM"))
        psD = ctx.enter_context(tc.tile_pool(name="psD", bufs=1, space="PSUM"))
        psA = ctx.enter_context(tc.tile_pool(name="psA", bufs=2, space="PSUM"))

        ident = p1.tile([P, P], bf, tag="ident")
        make_identity(nc, ident)
        ones_c = p1.tile([P, 1], bf, tag="ones_c")
        nc.vector.memset(ones_c[:], 1.0)
        ones_r = p1.tile([1, P], bf, tag="ones_r")
        nc.vector.memset(ones_r[:], 1.0)

        # ---- weight tiles (DMAs emitted in the startup sequence below,
        # interleaved with the first ht tile so PE starts early) ----
        wk_h = p1.tile([P, KS, HD], f8, tag="wk_h")
        wk_c = p1.tile([P, KS, 2, HD], f8, tag="wk_c")
        wv_h = p1.tile([P, KS, HD], f8, tag="wv_h")
        wv_c = p1.tile([P, KS, 2, HD], f8, tag="wv_c")
        wq_h = p1.tile([P, KS, ADIM], f8, tag="wq_h")
        wq_c = p1.tile([P, KS, 2, ADIM], f8, tag="wq_c")
        wo_h = p1.tile([P, QH, HIDDEN], f8, tag="wo_h")
        wo_l = p1.tile([P, QH, HIDDEN], f8, tag="wo_l")

        ht_cache = {}

        def load_ht(ti):
            ht = pht.tile([P, KS, 2, TW], f8, tag="ht")
            nc.sync.dma_start(ht[:], ht8[ti])
            ht_cache[ti] = ht
            return ht

        # per-seq resident k / v (rotating, sized for the longest seq)
        kT_seq = {}
        v_seq = {}

        # decode persistent tiles
        kT_dec = p1.tile([P, DECODE], bf, tag="kTdec")
        qdec_t = p1.tile([P, QH, DECODE], bf, tag="qdect")
        qdec_sb = p1.tile([P, P], bf, tag="qdec")
        vdt = p1.tile([DECODE, HD], bf, tag="vdt")
        odec_sb = p1.tile([P, HD], bf, tag="odec")
        aT_dec_h = p1.tile([P, QH, DECODE], f8, tag="aTdech")
        aT_dec_l = p1.tile([P, QH, DECODE], f8, tag="aTdecl")

        def proj_block(ps, wh, wc, ht, W):
            """fp8 DoubleRow projection of one 128-wide output block."""
            for j in range(KS // 2):
                nc.tensor.matmul(
                    ps[:, :W], lhsT=wh[:, 2 * j:2 * j + 2, :],
                    rhs=ht[:, 2 * j:2 * j + 2, 0, :W],
                    start=(j == 0), stop=False, perf_mode=DR)
            for kt in range(KS):
                nc.tensor.matmul(
                    ps[:, :W], lhsT=wc[:, kt, :, :],
                    rhs=ht[:, kt, :, :W],
                    start=False, stop=(kt == KS - 1), perf_mode=DR)

        def ph1_tile(ti, W, kT_dst, kcol0, v_dst, q_dst, qcol0):
            """Projections + rope for token tile ti (W valid cols).
            kT_dst[:, kcol0:+W] gets roped k;  q_dst [P, QH, *] gets roped
            q at qcol0;  v_dst: prefill -> v_nat [P, kt, HD+1] at k-tile
            kcol0//P (W=256), decode -> vdt [DECODE, HD] (W=32)."""
            t0 = ti * TW
            ht = ht_cache.pop(ti, None)
            if ht is None:
                ht = load_ht(ti)
                ht_cache.pop(ti)
            ct = pht.tile([P, TW], bf, tag="cos")
            st = pht.tile([P, TW], bf, tag="sin")
            nc.sync.dma_start(ct[:, :W], qcos[:, t0:t0 + W])
            nc.sync.dma_start(st[:, :W], qsin[:, t0:t0 + W])

            NB = QH + 1
            xq = pht.tile([P, NB, TW], bf, tag="xq")
            # k first (weights arrive first), then v, then q heads
            ps = psA.tile([P, 512], f32, tag="psA")
            proj_block(ps, wk_h, wk_c, ht, W)
            nc.scalar.activation(xq[:, QH, :W], ps[:, :W], AF.Copy)

            ps = psA.tile([P, 512], f32, tag="psA")
            proj_block(ps, wv_h, wv_c, ht, W)
            vt = pht.tile([P, TW], bf, tag="vt")
            nc.scalar.activation(vt[:, :W], ps[:, :W], AF.Copy,
                                 scale=1.0 / WS)
            if W == TW:
                for j in range(TW // P):
                    pst = psA.tile([P, P], bf, tag="psA")
                    nc.tensor.transpose(pst[:], vt[:, j * P:(j + 1) * P],
                                        ident[:])
                    nc.vector.tensor_copy(
                        out=v_dst[:, kcol0 // P + j, :HD], in_=pst[:])
            else:  # decode tile: W == 32
                pst = psA.tile([P, P], bf, tag="psA")
                nc.tensor.transpose(pst[:W, :], vt[:, :W], ident[:])
                nc.vector.tensor_copy(out=v_dst[:], in_=pst[:W, :])

            for m in range(QH):
                ps = psA.tile([P, 512], f32, tag="psA")
                proj_block(ps, wq_h[:, :, m * P:(m + 1) * P],
                           wq_c[:, :, :, m * P:(m + 1) * P], ht, W)
                nc.scalar.activation(xq[:, m, :W], ps[:, :W], AF.Copy)

            rotq = pht.tile([P, NB, TW], bf, tag="rotq")
            nc.gpsimd.dma_start(out=rotq[0:64, :, :W], in_=xq[64:128, :, :W])
            nc.gpsimd.dma_start(out=rotq[64:128, :, :W], in_=xq[0:64, :, :W])
            ct_b = ct[:, None, :W].to_broadcast((P, NB, W))
            st_b = st[:, None, :W].to_broadcast((P, NB, W))
            nc.vector.tensor_tensor(xq[:, :, :W], xq[:, :, :W], ct_b, ALU.mult)
            nc.vector.tensor_tensor(rotq[:, :, :W], rotq[:, :, :W], st_b,
                                    ALU.mult)
            nc.vector.tensor_tensor(q_dst[:, :, qcol0:qcol0 + W],
                                    xq[:, :QH, :W], rotq[:, :QH, :W], ALU.add)
            nc.vector.tensor_tensor(kT_dst[:, kcol0:kcol0 + W],
                                    xq[:, QH, :W], rotq[:, QH, :W], ALU.add)

        def ph2_panel(si, p, h, qT, aT_h, aT_l):
            """Attention for (seq si, panel p, head h): S^T orientation."""
            kT_sb = kT_seq[si]
            v_nat = v_seq[si]
            nck = 4 * p + 4
            oT_ps = psO.tile([P, PW], f32, tag="psO")
            den = psD.tile([1, PW], f32, tag="psD")

            stage = []   # chunks with pending den+PV (software pipeline)

            def drain_one():
                ct_, off_, w_, pt_ = stage.pop(0)
                nc.tensor.matmul(den[0:1, off_:off_ + w_],
                                 lhsT=ones_c[:], rhs=pt_[:, off_:off_ + w_],
                                 start=(ct_ == 0), stop=(ct_ == nck - 1))
                nc.tensor.matmul(oT_ps[:, off_:off_ + w_],
                                 lhsT=v_nat[:, ct_, :HD],
                                 rhs=pt_[:, off_:off_ + w_],
                                 start=(ct_ == 0), stop=(ct_ == nck - 1))

            for ct in range(nck):
                off = max(0, (ct - 4 * p)) * P
                w = PW - off
                sps = psS.tile([P, PW], f32, tag="psS")
                nc.tensor.matmul(sps[:, off:off + w],
                                 lhsT=kT_sb[:, ct * P:(ct + 1) * P],
                                 rhs=qT[:, h, off:PW],
                                 start=True, stop=True)
                pt = ppt.tile([P, PW], bf, tag="pt")
                nc.scalar.activation(pt[:, off:off + w], sps[:, off:off + w],
                                     AF.Exp, scale=SCALE)
                if ct >= 4 * p:   # diagonal block: mask c > q within 128 cols
                    nc.gpsimd.affine_select(
                        out=pt[:, off:off + P], in_=pt[:, off:off + P],
                        compare_op=ALU.is_ge, fill=0.0,
                        base=0, channel_multiplier=-1, pattern=[[1, P]])
                stage.append((ct, off, w, pt))
                if len(stage) >= 2:
                    drain_one()
            while stage:
                drain_one()

            # normalize: rr = 1/den, partition-broadcast via SWDGE, then
            # aT[:, h, :] = oT * rr_b  (single DVE op)
            rr = psm.tile([1, PW], bf, tag="rr")
            with nc.allow_low_precision(reason="softmax denom bcast in bf16"):
                nc.vector.reciprocal(rr[:], den[0:1, :])
            rr_ps = psA.tile([P, PW], f32, tag="psA")
            nc.tensor.matmul(rr_ps[:], lhsT=ones_r[:], rhs=rr[:],
                             start=True, stop=True)
            rrb = psm.tile([P, PW], bf, tag="rrb")
            nc.scalar.activation(rrb[:], rr_ps[:], AF.Copy)
            # normalized out -> bf16 t1 -> fp8 hi + residual lo
            t1 = pd1.tile([P, PW], bf, tag="t1")
            nc.vector.tensor_tensor(t1[:], oT_ps[:], rrb[:], ALU.mult)
            nc.scalar.activation(aT_h[:, h, :], t1[:], AF.Copy)
            nc.vector.tensor_tensor(aT_l[:, h, :], t1[:], aT_h[:, h, :],
                                    ALU.subtract)

        def ph4_panel(aT_h, aT_l, t0, W):
            """fp8 o_proj partial for W tokens at global t0.
            Wo'X ~= Wh.Xh + Wh.Xl + Wl.Xh, 3 DoubleRows per head-pair,
            emitted in <=256-col halves (DoubleRow moving limit)."""
            halves = [(c0, min(TW, W - c0)) for c0 in range(0, W, TW)]
            for g in range(8):
                for c0, cw in halves:
                    cs = slice(c0, c0 + cw)
                    omb = pmb.tile([P, 4, TW], bf, tag="omb")
                    for mi in range(4):
                        m = g * 4 + mi
                        ms = slice(m * P, (m + 1) * P)
                        ps = psA.tile([P, 512], f32, tag="psA")
                        first = True
                        for wsb, asb in ((wo_h, aT_h), (wo_h, aT_l),
                                         (wo_l, aT_h)):
                            for hp in (0, 2):
                                nc.tensor.matmul(
                                    ps[:, :cw],
                                    lhsT=wsb[:, hp:hp + 2, ms],
                                    rhs=asb[:, hp:hp + 2, cs],
                                    start=first,
                                    stop=(wsb is wo_l and hp == 2),
                                    perf_mode=DR)
                                first = False
                        if mi % 2 == 0:
                            nc.scalar.activation(omb[:, mi, :cw], ps[:, :cw],
                                                 AF.Copy, scale=1.0 / WS)
                        else:
                            nc.vector.tensor_scalar(
                                out=omb[:, mi, :cw], in0=ps[:, :cw],
                                scalar1=1.0 / WS, scalar2=None,
                                op0=ALU.mult)
                    nc.sync.dma_start(
                        outT_r[:, g * 4:(g + 1) * 4, t0 + c0:t0 + c0 + cw],
                        omb[:, :, :cw])

        # ---------------- decode helpers ----------------
        dec_tiles = {}

        def decode_dma(s):
            kd = pdec.tile([P, PAST], bf, tag="kd")
            nc.sync.dma_start(kd[:], kTc[s])
            vd = pvd.tile([P, NKT_D + 1, HD + 1], bf, tag="vd")
            nc.sync.dma_start(vd[:, :NKT_D, :], vcn[s])
            dec_tiles[s] = (kd, vd)

        def decode_compute(s):
            kd, vd = dec_tiles.pop(s)
            nc.gpsimd.dma_start(out=vd[0:1, NKT_D, :HD], in_=vdt[s:s + 1, :])
            nc.vector.memset(vd[0:1, NKT_D, HD:HD + 1], 1.0)

            stp = psS.tile([P, PW], f32, tag="psS")
            for kt in range(NKT_D):
                nc.tensor.matmul(
                    stp[:, kt * QH:(kt + 1) * QH],
                    lhsT=kd[:, kt * P:(kt + 1) * P],
                    rhs=qdec_sb[:, s * QH:(s + 1) * QH], start=True, stop=True)
            nc.tensor.matmul(
                stp[0:1, 64:68], lhsT=kT_dec[:, s:s + 1],
                rhs=qdec_sb[:, s * QH:(s + 1) * QH], start=True, stop=True)
            pt = pd1.tile([P, PW], bf, tag="ptd")
            nc.scalar.activation(pt[:, :64], stp[:, :64], AF.Exp, scale=SCALE)
            nc.scalar.activation(pt[0:1, 64:68], stp[0:1, 64:68], AF.Exp,
                                 scale=SCALE)

            ov = psA.tile([QH, HD + 1], f32, tag="psA")
            for kt in range(NKT_D):
                nc.tensor.matmul(
                    ov[:], lhsT=pt[:, kt * QH:(kt + 1) * QH],
                    rhs=vd[:, kt, :], start=(kt == 0), stop=False)
            nc.tensor.matmul(ov[:], lhsT=pt[0:1, 64:68],
                             rhs=vd[0:1, NKT_D, :], start=False, stop=True)
            r4 = pd1.tile([QH, 1], f32, tag="r4")
            nc.vector.reciprocal(r4[:], ov[:, HD:HD + 1])
            o4 = pd1.tile([QH, HD], bf, tag="o4")
            nc.vector.tensor_scalar_mul(o4[:], ov[:, :HD], r4[:])
            nc.gpsimd.dma_start(out=odec_sb[s * QH:(s + 1) * QH, :], in_=o4[:])

        # ================= emission =================
        def seq_tiles(si, p):
            s0, L = SEQ_BOUNDS[si]
            kT_sb = kT_seq[si]
            v_nat = v_seq[si]
            qT = qT_pan[(si, p)]
            for half in range(2):
                t0 = s0 + p * PW + half * TW
                ph1_tile(t0 // TW, TW, kT_sb, p * PW + half * TW,
                         v_nat, qT, half * TW)

        qT_pan = {}

        def alloc_seq(si):
            kT_seq[si] = pseq.tile([P, 2048], bf, tag="kT",
                                   name=f"kT{si}")
            v_seq[si] = pseq.tile([P, 2048 // P, HD + 1], bf, tag="vn",
                                  name=f"vn{si}")

        def alloc_panel(si, p):
            qT_pan[(si, p)] = ppan.tile([P, QH, PW], bf, tag="qT",
                                        name=f"qT{si}_{p}")

        # ---- startup: first ht tile DMA, then weights in need-order ----
        si0, p0, _ = PANELS[0]
        s00, _ = SEQ_BOUNDS[si0]
        first_ti = (s00 + p0 * PW) // TW
        load_ht(first_ti)
        nc.sync.dma_start(wk_h[:], whk[:])
        nc.sync.dma_start(wk_c[:], wck[:])
        nc.sync.dma_start(wv_h[:], whv[:])
        nc.sync.dma_start(wv_c[:], wcv[:])
        load_ht(first_ti + 1)
        nc.sync.dma_start(wq_h[:], whq[:])
        nc.sync.dma_start(wq_c[:], wcq[:])

        alloc_seq(si0)
        alloc_panel(si0, p0)
        seq_tiles(si0, p0)
        nc.sync.dma_start(wo_h[:], woh[:])
        nc.sync.dma_start(wo_l[:], wol[:])
        ph1_tile(NT - 1, DECODE, kT_dec, 0, vdt, qdec_t, 0)
        qd_r = qdec_sb.rearrange("p (s h) -> p s h", h=QH)
        for h in range(QH):
            nc.gpsimd.dma_start(out=qd_r[:, :, h], in_=qdec_t[:, h, :])

        dec_next_dma = 0
        dec_next_cmp = 0
        dec_out_done = False

        def decode_outputs():
            # decode outputs -> aT_dec hi/lo -> o_proj
            pst = psA.tile([P, P], bf, tag="psA")
            nc.tensor.transpose(pst[:], odec_sb[:], ident[:])
            ot = pd1.tile([P, P], bf, tag="otd")
            nc.vector.tensor_copy(out=ot[:], in_=pst[:])
            ot_h = pd1.tile([P, P], f8, tag="otdh")
            nc.scalar.activation(ot_h[:], ot[:], AF.Copy)
            ot_l = pd1.tile([P, P], f8, tag="otdl")
            nc.vector.tensor_tensor(ot_l[:], ot[:], ot_h[:], ALU.subtract)
            oh_r = ot_h.rearrange("d (s h) -> d s h", h=QH)
            ol_r = ot_l.rearrange("d (s h) -> d s h", h=QH)
            for h in range(QH):
                nc.gpsimd.dma_start(out=aT_dec_h[:, h, :], in_=oh_r[:, :, h])
                nc.gpsimd.dma_start(out=aT_dec_l[:, h, :], in_=ol_r[:, :, h])
            ph4_panel(aT_dec_h, aT_dec_l, DOFF, DECODE)

        def decode_slot():
            nonlocal dec_next_dma, dec_next_cmp, dec_out_done
            if dec_next_cmp < dec_next_dma:
                decode_compute(dec_next_cmp)
                dec_next_cmp += 1
            if dec_next_dma < DECODE:
                decode_dma(dec_next_dma)
                dec_next_dma += 1
            if dec_next_cmp == DECODE and not dec_out_done:
                dec_out_done = True
                decode_outputs()

        for step, (si, p, t0g) in enumerate(PANELS):
            # ph1 one panel ahead
            if step + 1 < len(PANELS):
                nsi, np_, _ = PANELS[step + 1]
                if np_ == 0:
                    alloc_seq(nsi)
                alloc_panel(nsi, np_)
                seq_tiles(nsi, np_)
            aT_h = ppan.tile([P, QH, PW], f8, tag="aTh", name=f"aTh{si}_{p}")
            aT_l = ppan.tile([P, QH, PW], f8, tag="aTl", name=f"aTl{si}_{p}")
            qT = qT_pan.pop((si, p))
            for h in range(QH):
                ph2_panel(si, p, h, qT, aT_h, aT_l)
                decode_slot()
            ph4_panel(aT_h, aT_l, t0g, PW)
            decode_slot()

        # remaining decode (normally already drained)
        while dec_next_cmp < DECODE:
            if dec_next_dma < DECODE:
                decode_dma(dec_next_dma)
                dec_next_dma += 1
            decode_compute(dec_next_cmp)
            dec_next_cmp += 1
        if not dec_out_done:
            decode_outputs()

    nc.compile()
    return nc


_NC = None


def _get_program():
    global _NC
    if _NC is None:
        _NC = build_program()
    return _NC


def _rope_tables():
    """cos/sin tables [128, T] with the 1/WS projection descale folded in."""
    inv_freq = 1.0 / (10000.0 ** (np.arange(0, HD, 2, dtype=np.float32) / HD))
    pos_q = np.concatenate(
        [np.arange(L, dtype=np.float32) for L in PREFILLS]
        + [np.full(DECODE, float(PAST), np.float32)])                 # [T]
    ang = np.outer(inv_freq, pos_q)                                   # [64, T]
    qcos = np.concatenate([np.cos(ang), np.cos(ang)], axis=0) / WS
    qsin = np.concatenate([-np.sin(ang), np.sin(ang)], axis=0) / WS
    return qcos.astype(BF16), qsin.astype(BF16)


def _split_w(wT):
    """wT [4096, M] f32 -> (wh [128, 32, M], wc [128, 32, 2, M]) fp8."""
    M = wT.shape[1]
    wp = (wT * WS).astype(np.float32)
    wh = wp.astype(E4M3)
    wl = (wp - wh.astype(np.float32)).astype(E4M3)
    wd = (wh.astype(np.float32) / WS).astype(E4M3)
    wh3 = np.ascontiguousarray(
        wh.reshape(KS, P, M).transpose(1, 0, 2))
    wc = np.ascontiguousarray(
        np.stack([wl.reshape(KS, P, M), wd.reshape(KS, P, M)],
                 axis=2).transpose(1, 0, 2, 3))
    return wh3, wc


def make_in_maps(hidden_states, wq, wk, wv, wo, kv_cache_k, kv_cache_v):
    hidden_states = np.asarray(hidden_states, np.float32)
    wq, wk, wv, wo = (np.asarray(a, np.float32) for a in (wq, wk, wv, wo))
    kv_cache_k = np.asarray(kv_cache_k, np.float32)
    kv_cache_v = np.asarray(kv_cache_v, np.float32)

    # hidden^T split into fp8 hi/lo, packed tile-major [NT,128,32,2,256]
    hT = hidden_states.T                                   # [4096, T]
    pad = NT * TW - T
    hTp = np.pad(hT, ((0, 0), (0, pad)))
    xh = hTp.astype(E4M3)
    xl = ((hTp - xh.astype(np.float32)) * WS).astype(E4M3)
    # [4096, NTT] -> [32, 128, NT, 256] -> [NT, 128, 32, 256]
    def pack(a):
        return a.reshape(KS, P, NT, TW).transpose(2, 1, 0, 3)
    ht8 = np.ascontiguousarray(
        np.stack([pack(xh), pack(xl)], axis=3))            # [NT,128,32,2,256]

    qcos, qsin = _rope_tables()

    # host-side RoPE of the k cache (reference semantics, fp32)
    inv_freq = 1.0 / (10000.0 ** (np.arange(0, HD, 2, dtype=np.float32) / HD))
    kpos = np.arange(PAST, dtype=np.float32)
    ang = np.outer(kpos, inv_freq)                          # [PAST, 64]
    cos = np.concatenate([np.cos(ang), np.cos(ang)], axis=1)[None, :, None, :]
    sin = np.concatenate([np.sin(ang), np.sin(ang)], axis=1)[None, :, None, :]
    rot = np.concatenate([-kv_cache_k[..., HD // 2:],
                          kv_cache_k[..., :HD // 2]], axis=-1)
    kroped = kv_cache_k * cos + rot * sin                   # [D, PAST, 8, HD]

    in_maps = []
    for c in range(NCORES):
        wh_q, wc_q = _split_w(wq[c * ADIM:(c + 1) * ADIM, :].T)
        wh_k, wc_k = _split_w(wk[c * HD:(c + 1) * HD, :].T)
        wh_v, wc_v = _split_w(wv[c * HD:(c + 1) * HD, :].T)
        wot = wo[:, c * ADIM:(c + 1) * ADIM].T.reshape(QH, P, HIDDEN) \
            .transpose(1, 0, 2) * WS                        # [128, 4, 4096]
        wo_hq = wot.astype(E4M3)
        wo_lq = (wot - wo_hq.astype(np.float32)).astype(E4M3)
        wo_hq = np.ascontiguousarray(wo_hq)
        wo_lq = np.ascontiguousarray(wo_lq)
        kTcc = np.ascontiguousarray(
            kroped[:, :, c, :].transpose(0, 2, 1).astype(BF16))  # [D,128,PAST]
        # v cache -> [D, 128, 16, 129] with ones column baked in
        vcc = kv_cache_v[:, :, c, :].reshape(DECODE, NKT_D, P, HD)
        vcc = vcc.transpose(0, 2, 1, 3)                     # [D, 128, 16, HD]
        vcn = np.concatenate(
            [vcc, np.ones((DECODE, P, NKT_D, 1), np.float32)], axis=3)
        vcn = np.ascontiguousarray(vcn.astype(BF16))
        in_maps.append({
            "ht8": ht8, "whq": wh_q, "wcq": wc_q, "whk": wh_k, "wck": wc_k,
            "whv": wh_v, "wcv": wc_v, "woh": wo_hq, "wol": wo_lq, "kTc": kTcc, "vcn": vcn,
            "qcos": qcos, "qsin": qsin,
        })
    return in_maps


def combine_outputs(results):
    acc = np.zeros((HIDDEN, T), np.float32)
    for c in range(NCORES):
        acc += results[c]["outT"].astype(np.float32)
    return np.ascontiguousarray(acc.T)


def kernel(hidden_states, wq, wk, wv, wo, kv_cache_k, kv_cache_v):
    from concourse.bass_utils import run_bass_kernel_spmd

    nc = _get_program()
    in_maps = make_in_maps(hidden_states, wq, wk, wv, wo, kv_cache_k,
                           kv_cache_v)
    res = run_bass_kernel_spmd(nc, in_maps, core_ids=list(range(NCORES)))
    return combine_outputs(res.results)


# revision 27
# speedup vs baseline: 1.2263x; 1.0190x over previous
"""Trainium2 Bass kernel for nn_LlamaAttention_61495341744411.

Sharding: tensor-parallel over heads across 8 NeuronCores.
  core c: q heads [4c, 4c+4), kv head c, wo cols [512c, 512c+512).
  Each core computes a full-token partial of out^T; host sums partials.

v2 design (per core, single SPMD program):
  - q/k/v projections in fp8(e4m3) DoubleRow with hi/lo error correction:
    X = Xh + Xl/32, W' = 32W = Wh + Wl;  W'X ~= Wh.Xh + Wl.Xh + (Wh/32).Xl
    computed as 3 DoubleRow matmuls per 2 k-tiles (1.33x bf16 FLOP rate,
    near-bf16 accuracy; validated vs reference in numpy).  The /32
    prescale is folded into the RoPE cos/sin tables (q,k) and the v copy.
  - attention computed in score-transposed orientation (S^T = K^T.q panels
    of 512), eliminating all P-transposes; PV accumulates oT directly;
    softmax denominators via ones-vector matmul; normalization by a
    rank-1 PE broadcast of 1/denom + one DVE multiply per (panel, head).
  - kv-cache K is RoPE'd on the host; decode attention is interleaved
    across the prefill panels so its DMA fully overlaps compute.
  - o_proj per panel from SBUF-resident attn outputs (no DRAM spills).
"""
import sys

if "/opt/trn_rl_repo" not in sys.path:
    sys.path.insert(0, "/opt/trn_rl_repo")

import numpy as np
import ml_dtypes

BF16 = ml_dtypes.bfloat16
E4M3 = ml_dtypes.float8_e4m3

PREFILLS = [1024, 1536, 2048, 512]
DOFF = sum(PREFILLS)            # 5120
DECODE = 32
PAST = 2048
HIDDEN = 4096
NQ, NKV, HD = 32, 8, 128
G = NQ // NKV                   # 4
T = DOFF + DECODE               # 5152
SCALE = 1.0 / float(np.sqrt(HD))
NCORES = 8
QH = NQ // NCORES               # 4 q heads per core
ADIM = QH * HD                  # 512
KS = HIDDEN // 128              # 32 contraction k-tiles
P = 128
TW = 256                        # projection token-tile width
PW = 512                        # attention q-panel width
NT = (T + TW - 1) // TW         # 21 token tiles (last = decode, 32 valid)
NKT_D = PAST // P               # 16 decode cache k-tiles
WS = 32.0                       # weight prescale

SEQ_BOUNDS = []
_off = 0
for _L in PREFILLS:
    SEQ_BOUNDS.append((_off, _L))
    _off += _L

# (si, panel, global t0) for every 512-token prefill panel.
# Short seq first so the largest panel (max ILP) lands last and hides
# the decode/o_proj tail.
SEQ_ORDER = [3, 0, 1, 2]
PANELS = []
for _si in SEQ_ORDER:
    _s0, _L = SEQ_BOUNDS[_si]
    for _p in range(_L // PW):
        PANELS.append((_si, _p, _s0 + _p * PW))


def build_program():
    import concourse.mybir as mybir
    import concourse.tile as tile
    from concourse import bacc
    from concourse.masks import make_identity
    from contextlib import ExitStack

    dt = mybir.dt
    AF = mybir.ActivationFunctionType
    ALU = mybir.AluOpType
    DR = mybir.MatmulPerfMode.DoubleRow
    f32 = dt.float32
    bf = dt.bfloat16
    f8 = dt.float8e4

    nc = bacc.Bacc(None, target_bir_lowering=False, debug=False)

    ht8 = nc.dram_tensor("ht8", [NT, P, KS, 2, TW], f8, kind="ExternalInput")
    whq = nc.dram_tensor("whq", [P, KS, ADIM], f8, kind="ExternalInput")
    wcq = nc.dram_tensor("wcq", [P, KS, 2, ADIM], f8, kind="ExternalInput")
    whk = nc.dram_tensor("whk", [P, KS, HD], f8, kind="ExternalInput")
    wck = nc.dram_tensor("wck", [P, KS, 2, HD], f8, kind="ExternalInput")
    whv = nc.dram_tensor("whv", [P, KS, HD], f8, kind="ExternalInput")
    wcv = nc.dram_tensor("wcv", [P, KS, 2, HD], f8, kind="ExternalInput")
    woh = nc.dram_tensor("woh", [P, QH, HIDDEN], f8, kind="ExternalInput")
    wol = nc.dram_tensor("wol", [P, QH, HIDDEN], f8, kind="ExternalInput")
    kTc = nc.dram_tensor("kTc", [DECODE, HD, PAST], bf, kind="ExternalInput")
    vcn = nc.dram_tensor("vcn", [DECODE, P, NKT_D, HD + 1], bf,
                         kind="ExternalInput")
    qcos = nc.dram_tensor("qcos", [HD, T], bf, kind="ExternalInput")
    qsin = nc.dram_tensor("qsin", [HD, T], bf, kind="ExternalInput")
    outT = nc.dram_tensor("outT", [HIDDEN, T], bf, kind="ExternalOutput")
    outT_r = outT.rearrange("(o p) t -> p o t", p=P)    # [128, 32, T]

    with ExitStack() as ctx:
        tc = ctx.enter_context(tile.TileContext(nc))
        p1 = ctx.enter_context(tc.tile_pool(name="p1", bufs=1))
        pseq = ctx.enter_context(tc.tile_pool(name="pseq", bufs=2))
        ppan = ctx.enter_context(tc.tile_pool(name="ppan", bufs=2))
        pht = ctx.enter_context(tc.tile_pool(name="pht", bufs=2))
        ppt = ctx.enter_context(tc.tile_pool(name="ppt", bufs=4))
        pdec = ctx.enter_context(tc.tile_pool(name="pdec", bufs=1))
        pvd = ctx.enter_context(tc.tile_pool(name="pvd", bufs=1))
        pd1 = ctx.enter_context(tc.tile_pool(name="pd1", bufs=1))
        pmb = ctx.enter_context(tc.tile_pool(name="pmb", bufs=2))
        psm = ctx.enter_context(tc.tile_pool(name="psm", bufs=2))
        psS = ctx.enter_context(tc.tile_pool(name="psS", bufs=3, space="PSUM"))
        psO = ctx.enter_context(tc.tile_pool(name="psO", bufs=2, space="PSUM"))
        psD = ctx.enter_context(tc.tile_pool(name="psD", bufs=1, space="PSUM"))
        psA = ctx.enter_context(tc.tile_pool(name="psA", bufs=2, space="PSUM"))

        ident = p1.tile([P, P], bf, tag="ident")
        make_identity(nc, ident)
        ones_c = p1.tile([P, 1], bf, tag="ones_c")
        nc.vector.memset(ones_c[:], 1.0)
        ones_r = p1.tile([1, P], bf, tag="ones_r")
        nc.vector.memset(ones_r[:], 1.0)
        # upper-triangular (keep col >= row) causal mask for diagonal chunks
        mask_tri = p1.tile([P, P], bf, tag="mask_tri")
        nc.vector.memset(mask_tri[:], 1.0)
        nc.gpsimd.affine_select(
            out=mask_tri[:], in_=mask_tri[:], compare_op=ALU.is_ge,
            fill=0.0, base=0, channel_multiplier=-1, pattern=[[1, P]])

        # ---- weight tiles (DMAs emitted in the startup sequence below,
        # interleaved with the first ht tile so PE starts early) ----
        wk_h = p1.tile([P, KS, HD], f8, tag="wk_h")
        wk_c = p1.tile([P, KS, 2, HD], f8, tag="wk_c")
        wv_h = p1.tile([P, KS, HD], f8, tag="wv_h")
        wv_c = p1.tile([P, KS, 2, HD], f8, tag="wv_c")
        wq_h = p1.tile([P, KS, ADIM], f8, tag="wq_h")
        wq_c = p1.tile([P, KS, 2, ADIM], f8, tag="wq_c")
        wo_h = p1.tile([P, QH, HIDDEN], f8, tag="wo_h")
        wo_l = p1.tile([P, QH, HIDDEN], f8, tag="wo_l")

        ht_cache = {}

        def load_ht(ti):
            ht = pht.tile([P, KS, 2, TW], f8, tag="ht")
            nc.sync.dma_start(ht[:], ht8[ti])
            ht_cache[ti] = ht
            return ht

        # per-seq resident k / v (rotating, sized for the longest seq)
        kT_seq = {}
        v_seq = {}

        # decode persistent tiles
        kT_dec = p1.tile([P, DECODE], bf, tag="kTdec")
        qdec_t = p1.tile([P, QH, DECODE], bf, tag="qdect")
        qdec_sb = p1.tile([P, P], bf, tag="qdec")
        vdt = p1.tile([DECODE, HD], bf, tag="vdt")
        odec_sb = p1.tile([P, HD], bf, tag="odec")
        aT_dec_h = p1.tile([P, QH, DECODE], f8, tag="aTdech")
        aT_dec_l = p1.tile([P, QH, DECODE], f8, tag="aTdecl")

        def proj_block(ps, wh, wc, ht, W):
            """fp8 DoubleRow projection of one 128-wide output block."""
            for j in range(KS // 2):
                nc.tensor.matmul(
                    ps[:, :W], lhsT=wh[:, 2 * j:2 * j + 2, :],
                    rhs=ht[:, 2 * j:2 * j + 2, 0, :W],
                    start=(j == 0), stop=False, perf_mode=DR)
            for kt in range(KS):
                nc.tensor.matmul(
                    ps[:, :W], lhsT=wc[:, kt, :, :],
                    rhs=ht[:, kt, :, :W],
                    start=False, stop=(kt == KS - 1), perf_mode=DR)

        def ph1_tile(ti, W, kT_dst, kcol0, v_dst, q_dst, qcol0):
            """Projections + rope for token tile ti (W valid cols).
            kT_dst[:, kcol0:+W] gets roped k;  q_dst [P, QH, *] gets roped
            q at qcol0;  v_dst: prefill -> v_nat [P, kt, HD+1] at k-tile
            kcol0//P (W=256), decode -> vdt [DECODE, HD] (W=32)."""
            t0 = ti * TW
            ht = ht_cache.pop(ti, None)
            if ht is None:
                ht = load_ht(ti)
                ht_cache.pop(ti)
            ct = pht.tile([P, TW], bf, tag="cos")
            st = pht.tile([P, TW], bf, tag="sin")
            nc.sync.dma_start(ct[:, :W], qcos[:, t0:t0 + W])
            nc.sync.dma_start(st[:, :W], qsin[:, t0:t0 + W])

            NB = QH + 1
            xq = pht.tile([P, NB, TW], bf, tag="xq")
            # k first (weights arrive first), then v, then q heads
            ps = psA.tile([P, 512], f32, tag="psA")
            proj_block(ps, wk_h, wk_c, ht, W)
            nc.scalar.activation(xq[:, QH, :W], ps[:, :W], AF.Copy)

            ps = psA.tile([P, 512], f32, tag="psA")
            proj_block(ps, wv_h, wv_c, ht, W)
            vt = pht.tile([P, TW], bf, tag="vt")
            nc.scalar.activation(vt[:, :W], ps[:, :W], AF.Copy,
                                 scale=1.0 / WS)
            if W == TW:
                for j in range(TW // P):
                    pst = psA.tile([P, P], bf, tag="psA")
                    nc.tensor.transpose(pst[:], vt[:, j * P:(j + 1) * P],
                                        ident[:])
                    nc.vector.tensor_copy(
                        out=v_dst[:, kcol0 // P + j, :HD], in_=pst[:])
            else:  # decode tile: W == 32
                pst = psA.tile([P, P], bf, tag="psA")
                nc.tensor.transpose(pst[:W, :], vt[:, :W], ident[:])
                nc.vector.tensor_copy(out=v_dst[:], in_=pst[:W, :])

            for m in range(QH):
                ps = psA.tile([P, 512], f32, tag="psA")
                proj_block(ps, wq_h[:, :, m * P:(m + 1) * P],
                           wq_c[:, :, :, m * P:(m + 1) * P], ht, W)
                nc.scalar.activation(xq[:, m, :W], ps[:, :W], AF.Copy)

            rotq = pht.tile([P, NB, TW], bf, tag="rotq")
            nc.gpsimd.dma_start(out=rotq[0:64, :, :W], in_=xq[64:128, :, :W])
            nc.gpsimd.dma_start(out=rotq[64:128, :, :W], in_=xq[0:64, :, :W])
            ct_b = ct[:, None, :W].to_broadcast((P, NB, W))
            st_b = st[:, None, :W].to_broadcast((P, NB, W))
            nc.vector.tensor_tensor(xq[:, :, :W], xq[:, :, :W], ct_b, ALU.mult)
            nc.vector.tensor_tensor(rotq[:, :, :W], rotq[:, :, :W], st_b,
                                    ALU.mult)
            nc.vector.tensor_tensor(q_dst[:, :, qcol0:qcol0 + W],
                                    xq[:, :QH, :W], rotq[:, :QH, :W], ALU.add)
            nc.vector.tensor_tensor(kT_dst[:, kcol0:kcol0 + W],
                                    xq[:, QH, :W], rotq[:, QH, :W], ALU.add)

        def ph2_panel(si, p, h, qT, aT_h, aT_l):
            """Attention for (seq si, panel p, head h): S^T orientation.
            Interleaves pending o_proj work to fill PE stalls."""
            kT_sb = kT_seq[si]
            v_nat = v_seq[si]
            nck = 4 * p + 4
            oT_ps = psO.tile([P, PW], f32, tag="psO")
            den = psD.tile([1, PW], f32, tag="psD")

            stage = []   # chunks with pending den+PV (software pipeline)

            def drain_one():
                ct_, off_, w_, pt_ = stage.pop(0)
                nc.tensor.matmul(den[0:1, off_:off_ + w_],
                                 lhsT=ones_c[:], rhs=pt_[:, off_:off_ + w_],
                                 start=(ct_ == 0), stop=(ct_ == nck - 1))
                nc.tensor.matmul(oT_ps[:, off_:off_ + w_],
                                 lhsT=v_nat[:, ct_, :HD],
                                 rhs=pt_[:, off_:off_ + w_],
                                 start=(ct_ == 0), stop=(ct_ == nck - 1))

            for ct in range(nck):
                off = max(0, (ct - 4 * p)) * P
                w = PW - off
                sps = psS.tile([P, PW], f32, tag="psS")
                nc.tensor.matmul(sps[:, off:off + w],
                                 lhsT=kT_sb[:, ct * P:(ct + 1) * P],
                                 rhs=qT[:, h, off:PW],
                                 start=True, stop=True)
                pt = ppt.tile([P, PW], bf, tag="pt")
                nc.scalar.activation(pt[:, off:off + w], sps[:, off:off + w],
                                     AF.Exp, scale=SCALE)
                if ct >= 4 * p:   # diagonal block: mask c > q within 128 cols
                    nc.vector.tensor_tensor(pt[:, off:off + P],
                                            pt[:, off:off + P], mask_tri[:],
                                            ALU.mult)
                stage.append((ct, off, w, pt))
                if len(stage) >= 3:
                    drain_one()
                drain_ph4(1)
            while stage:
                drain_one()

            # normalize: rr = 1/den, partition-broadcast via SWDGE, then
            # aT[:, h, :] = oT * rr_b  (single DVE op)
            rr = psm.tile([1, PW], bf, tag="rr")
            with nc.allow_low_precision(reason="softmax denom bcast in bf16"):
                nc.vector.reciprocal(rr[:], den[0:1, :])
            rr_ps = psA.tile([P, PW], f32, tag="psA")
            nc.tensor.matmul(rr_ps[:], lhsT=ones_r[:], rhs=rr[:],
                             start=True, stop=True)
            rrb = psm.tile([P, PW], bf, tag="rrb")
            nc.vector.tensor_copy(out=rrb[:], in_=rr_ps[:])
            # normalized out -> bf16 t1 -> fp8 hi + residual lo
            t1 = pd1.tile([P, PW], bf, tag="t1")
            nc.vector.tensor_tensor(t1[:], oT_ps[:], rrb[:], ALU.mult)
            nc.scalar.activation(aT_h[:, h, :], t1[:], AF.Copy)
            nc.vector.tensor_tensor(aT_l[:, h, :], t1[:], aT_h[:, h, :],
                                    ALU.subtract)

        def ph4_unit(g, c0, cw, aT_h, aT_l, t0):
            """One o_proj unit: 4 output m-blocks x one 256-col half.
            Wo'X ~= Wh.Xh + Wh.Xl + Wl.Xh, 3 DoubleRows per head-pair."""
            cs = slice(c0, c0 + cw)
            omb = pmb.tile([P, 4, TW], bf, tag="omb")
            for mi in range(4):
                m = g * 4 + mi
                ms = slice(m * P, (m + 1) * P)
                ps = psA.tile([P, 512], f32, tag="psA")
                first = True
                for wsb, asb in ((wo_h, aT_h), (wo_h, aT_l),
                                 (wo_l, aT_h)):
                    for hp in (0, 2):
                        nc.tensor.matmul(
                            ps[:, :cw],
                            lhsT=wsb[:, hp:hp + 2, ms],
                            rhs=asb[:, hp:hp + 2, cs],
                            start=first,
                            stop=(wsb is wo_l and hp == 2),
                            perf_mode=DR)
                        first = False
                if mi % 2 == 0:
                    nc.scalar.activation(omb[:, mi, :cw], ps[:, :cw],
                                         AF.Copy, scale=1.0 / WS)
                else:
                    nc.vector.tensor_scalar(
                        out=omb[:, mi, :cw], in0=ps[:, :cw],
                        scalar1=1.0 / WS, scalar2=None, op0=ALU.mult)
            nc.sync.dma_start(
                outT_r[:, g * 4:(g + 1) * 4, t0 + c0:t0 + c0 + cw],
                omb[:, :, :cw])

        ph4_pending = []

        def drain_ph4(n):
            for _ in range(min(n, len(ph4_pending))):
                ph4_pending.pop(0)()

        def ph4_panel(aT_h, aT_l, t0, W, queue=False):
            halves = [(c0, min(TW, W - c0)) for c0 in range(0, W, TW)]
            for g in range(8):
                for c0, cw in halves:
                    if queue:
                        ph4_pending.append(
                            lambda g=g, c0=c0, cw=cw, ah=aT_h, al=aT_l,
                            t0=t0: ph4_unit(g, c0, cw, ah, al, t0))
                    else:
                        ph4_unit(g, c0, cw, aT_h, aT_l, t0)

        # ---------------- decode helpers ----------------
        dec_tiles = {}

        def decode_dma(s):
            kd = pdec.tile([P, PAST], bf, tag="kd")
            nc.sync.dma_start(kd[:], kTc[s])
            vd = pvd.tile([P, NKT_D + 1, HD + 1], bf, tag="vd")
            nc.sync.dma_start(vd[:, :NKT_D, :], vcn[s])
            dec_tiles[s] = (kd, vd)

        def decode_compute(s):
            kd, vd = dec_tiles.pop(s)
            nc.gpsimd.dma_start(out=vd[0:1, NKT_D, :HD], in_=vdt[s:s + 1, :])
            nc.vector.memset(vd[0:1, NKT_D, HD:HD + 1], 1.0)

            stp = psS.tile([P, PW], f32, tag="psS")
            for kt in range(NKT_D):
                nc.tensor.matmul(
                    stp[:, kt * QH:(kt + 1) * QH],
                    lhsT=kd[:, kt * P:(kt + 1) * P],
                    rhs=qdec_sb[:, s * QH:(s + 1) * QH], start=True, stop=True)
            nc.tensor.matmul(
                stp[0:1, 64:68], lhsT=kT_dec[:, s:s + 1],
                rhs=qdec_sb[:, s * QH:(s + 1) * QH], start=True, stop=True)
            pt = pd1.tile([P, PW], bf, tag="ptd")
            nc.scalar.activation(pt[:, :64], stp[:, :64], AF.Exp, scale=SCALE)
            nc.scalar.activation(pt[0:1, 64:68], stp[0:1, 64:68], AF.Exp,
                                 scale=SCALE)

            ov = psA.tile([QH, HD + 1], f32, tag="psA")
            for kt in range(NKT_D):
                nc.tensor.matmul(
                    ov[:], lhsT=pt[:, kt * QH:(kt + 1) * QH],
                    rhs=vd[:, kt, :], start=(kt == 0), stop=False)
            nc.tensor.matmul(ov[:], lhsT=pt[0:1, 64:68],
                             rhs=vd[0:1, NKT_D, :], start=False, stop=True)
            r4 = pd1.tile([QH, 1], f32, tag="r4")
            nc.vector.reciprocal(r4[:], ov[:, HD:HD + 1])
            o4 = pd1.tile([QH, HD], bf, tag="o4")
            nc.vector.tensor_scalar_mul(o4[:], ov[:, :HD], r4[:])
            nc.gpsimd.dma_start(out=odec_sb[s * QH:(s + 1) * QH, :], in_=o4[:])

        # ================= emission =================
        def seq_tiles(si, p):
            s0, L = SEQ_BOUNDS[si]
            kT_sb = kT_seq[si]
            v_nat = v_seq[si]
            qT = qT_pan[(si, p)]
            for half in range(2):
                t0 = s0 + p * PW + half * TW
                ph1_tile(t0 // TW, TW, kT_sb, p * PW + half * TW,
                         v_nat, qT, half * TW)

        qT_pan = {}

        def alloc_seq(si):
            kT_seq[si] = pseq.tile([P, 2048], bf, tag="kT",
                                   name=f"kT{si}")
            v_seq[si] = pseq.tile([P, 2048 // P, HD + 1], bf, tag="vn",
                                  name=f"vn{si}")

        def alloc_panel(si, p):
            qT_pan[(si, p)] = ppan.tile([P, QH, PW], bf, tag="qT",
                                        name=f"qT{si}_{p}")

        # ---- startup: first ht tile DMA, then weights in need-order ----
        si0, p0, _ = PANELS[0]
        s00, _ = SEQ_BOUNDS[si0]
        first_ti = (s00 + p0 * PW) // TW
        load_ht(first_ti)
        nc.sync.dma_start(wk_h[:], whk[:])
        nc.sync.dma_start(wk_c[:], wck[:])
        nc.sync.dma_start(wv_h[:], whv[:])
        nc.sync.dma_start(wv_c[:], wcv[:])
        load_ht(first_ti + 1)
        nc.sync.dma_start(wq_h[:], whq[:])
        nc.sync.dma_start(wq_c[:], wcq[:])

        alloc_seq(si0)
        alloc_panel(si0, p0)
        seq_tiles(si0, p0)
        nc.sync.dma_start(wo_h[:], woh[:])
        nc.sync.dma_start(wo_l[:], wol[:])
        ph1_tile(NT - 1, DECODE, kT_dec, 0, vdt, qdec_t, 0)
        qd_r = qdec_sb.rearrange("p (s h) -> p s h", h=QH)
        for h in range(QH):
            nc.gpsimd.dma_start(out=qd_r[:, :, h], in_=qdec_t[:, h, :])

        dec_next_dma = 0
        dec_next_cmp = 0
        dec_out_done = False

        def decode_outputs():
            # decode outputs -> aT_dec hi/lo -> o_proj
            pst = psA.tile([P, P], bf, tag="psA")
            nc.tensor.transpose(pst[:], odec_sb[:], ident[:])
            ot = pd1.tile([P, P], bf, tag="otd")
            nc.vector.tensor_copy(out=ot[:], in_=pst[:])
            ot_h = pd1.tile([P, P], f8, tag="otdh")
            nc.scalar.activation(ot_h[:], ot[:], AF.Copy)
            ot_l = pd1.tile([P, P], f8, tag="otdl")
            nc.vector.tensor_tensor(ot_l[:], ot[:], ot_h[:], ALU.subtract)
            oh_r = ot_h.rearrange("d (s h) -> d s h", h=QH)
            ol_r = ot_l.rearrange("d (s h) -> d s h", h=QH)
            for h in range(QH):
                nc.gpsimd.dma_start(out=aT_dec_h[:, h, :], in_=oh_r[:, :, h])
                nc.gpsimd.dma_start(out=aT_dec_l[:, h, :], in_=ol_r[:, :, h])
            ph4_panel(aT_dec_h, aT_dec_l, DOFF, DECODE)

        def decode_slot():
            nonlocal dec_next_dma, dec_next_cmp, dec_out_done
            if dec_next_cmp < dec_next_dma:
                decode_compute(dec_next_cmp)
                dec_next_cmp += 1
            if dec_next_dma < DECODE:
                decode_dma(dec_next_dma)
                dec_next_dma += 1
            if dec_next_cmp == DECODE and not dec_out_done:
                dec_out_done = True
                decode_outputs()

        for step, (si, p, t0g) in enumerate(PANELS):
            # ph1 one panel ahead
            if step + 1 < len(PANELS):
                nsi, np_, _ = PANELS[step + 1]
                if np_ == 0:
                    alloc_seq(nsi)
                alloc_panel(nsi, np_)
                seq_tiles(nsi, np_)
            aT_h = ppan.tile([P, QH, PW], f8, tag="aTh", name=f"aTh{si}_{p}")
            aT_l = ppan.tile([P, QH, PW], f8, tag="aTl", name=f"aTl{si}_{p}")
            qT = qT_pan.pop((si, p))
            for h in range(QH):
                ph2_panel(si, p, h, qT, aT_h, aT_l)
                decode_slot()
            drain_ph4(len(ph4_pending))   # flush previous panel leftovers
            ph4_panel(aT_h, aT_l, t0g, PW, queue=True)
            decode_slot()
        drain_ph4(len(ph4_pending))

        # remaining decode (normally already drained)
        while dec_next_cmp < DECODE:
            if dec_next_dma < DECODE:
                decode_dma(dec_next_dma)
                dec_next_dma += 1
            decode_compute(dec_next_cmp)
            dec_next_cmp += 1
        if not dec_out_done:
            decode_outputs()

    nc.compile()
    return nc


_NC = None


def _get_program():
    global _NC
    if _NC is None:
        _NC = build_program()
    return _NC


def _rope_tables():
    """cos/sin tables [128, T] with the 1/WS projection descale folded in."""
    inv_freq = 1.0 / (10000.0 ** (np.arange(0, HD, 2, dtype=np.float32) / HD))
    pos_q = np.concatenate(
        [np.arange(L, dtype=np.float32) for L in PREFILLS]
        + [np.full(DECODE, float(PAST), np.float32)])                 # [T]
    ang = np.outer(inv_freq, pos_q)                                   # [64, T]
    qcos = np.concatenate([np.cos(ang), np.cos(ang)], axis=0) / WS
    qsin = np.concatenate([-np.sin(ang), np.sin(ang)], axis=0) / WS
    return qcos.astype(BF16), qsin.astype(BF16)


def _split_w(wT):
    """wT [4096, M] f32 -> (wh [128, 32, M], wc [128, 32, 2, M]) fp8."""
    M = wT.shape[1]
    wp = (wT * WS).astype(np.float32)
    wh = wp.astype(E4M3)
    wl = (wp - wh.astype(np.float32)).astype(E4M3)
    wd = (wh.astype(np.float32) / WS).astype(E4M3)
    wh3 = np.ascontiguousarray(
        wh.reshape(KS, P, M).transpose(1, 0, 2))
    wc = np.ascontiguousarray(
        np.stack([wl.reshape(KS, P, M), wd.reshape(KS, P, M)],
                 axis=2).transpose(1, 0, 2, 3))
    return wh3, wc


def make_in_maps(hidden_states, wq, wk, wv, wo, kv_cache_k, kv_cache_v):
    hidden_states = np.asarray(hidden_states, np.float32)
    wq, wk, wv, wo = (np.asarray(a, np.float32) for a in (wq, wk, wv, wo))
    kv_cache_k = np.asarray(kv_cache_k, np.float32)
    kv_cache_v = np.asarray(kv_cache_v, np.float32)

    # hidden^T split into fp8 hi/lo, packed tile-major [NT,128,32,2,256]
    hT = hidden_states.T                                   # [4096, T]
    pad = NT * TW - T
    hTp = np.pad(hT, ((0, 0), (0, pad)))
    xh = hTp.astype(E4M3)
    xl = ((hTp - xh.astype(np.float32)) * WS).astype(E4M3)
    # [4096, NTT] -> [32, 128, NT, 256] -> [NT, 128, 32, 256]
    def pack(a):
        return a.reshape(KS, P, NT, TW).transpose(2, 1, 0, 3)
    ht8 = np.ascontiguousarray(
        np.stack([pack(xh), pack(xl)], axis=3))            # [NT,128,32,2,256]

    qcos, qsin = _rope_tables()

    # host-side RoPE of the k cache (reference semantics, fp32)
    inv_freq = 1.0 / (10000.0 ** (np.arange(0, HD, 2, dtype=np.float32) / HD))
    kpos = np.arange(PAST, dtype=np.float32)
    ang = np.outer(kpos, inv_freq)                          # [PAST, 64]
    cos = np.concatenate([np.cos(ang), np.cos(ang)], axis=1)[None, :, None, :]
    sin = np.concatenate([np.sin(ang), np.sin(ang)], axis=1)[None, :, None, :]
    rot = np.concatenate([-kv_cache_k[..., HD // 2:],
                          kv_cache_k[..., :HD // 2]], axis=-1)
    kroped = kv_cache_k * cos + rot * sin                   # [D, PAST, 8, HD]

    in_maps = []
    for c in range(NCORES):
        wh_q, wc_q = _split_w(wq[c * ADIM:(c + 1) * ADIM, :].T)
        wh_k, wc_k = _split_w(wk[c * HD:(c + 1) * HD, :].T)
        wh_v, wc_v = _split_w(wv[c * HD:(c + 1) * HD, :].T)
        wot = wo[:, c * ADIM:(c + 1) * ADIM].T.reshape(QH, P, HIDDEN) \
            .transpose(1, 0, 2) * WS                        # [128, 4, 4096]
        wo_hq = wot.astype(E4M3)
        wo_lq = (wot - wo_hq.astype(np.float32)).astype(E4M3)
        wo_hq = np.ascontiguousarray(wo_hq)
        wo_lq = np.ascontiguousarray(wo_lq)
        kTcc = np.ascontiguousarray(
            kroped[:, :, c, :].transpose(0, 2, 1).astype(BF16))  # [D,128,PAST]
        # v cache -> [D, 128, 16, 129] with ones column baked in
        vcc = kv_cache_v[:, :, c, :].reshape(DECODE, NKT_D, P, HD)
        vcc = vcc.transpose(0, 2, 1, 3)                     # [D, 128, 16, HD]
        vcn = np.concatenate(
            [vcc, np.ones((DECODE, P, NKT_D, 1), np.float32)], axis=3)
        vcn = np.ascontiguousarray(vcn.astype(BF16))
        in_maps.append({
            "ht8": ht8, "whq": wh_q, "wcq": wc_q, "whk": wh_k, "wck": wc_k,
            "whv": wh_v, "wcv": wc_v, "woh": wo_hq, "wol": wo_lq, "kTc": kTcc, "vcn": vcn,
            "qcos": qcos, "qsin": qsin,
        })
    return in_maps


def combine_outputs(results):
    acc = np.zeros((HIDDEN, T), np.float32)
    for c in range(NCORES):
        acc += results[c]["outT"].astype(np.float32)
    return np.ascontiguousarray(acc.T)


def kernel(hidden_states, wq, wk, wv, wo, kv_cache_k, kv_cache_v):
    from concourse.bass_utils import run_bass_kernel_spmd

    nc = _get_program()
    in_maps = make_in_maps(hidden_states, wq, wk, wv, wo, kv_cache_k,
                           kv_cache_v)
    res = run_bass_kernel_spmd(nc, in_maps, core_ids=list(range(NCORES)))
    return combine_outputs(res.results)
